# revision 1
# baseline (speedup 1.0000x reference)
"""Trainium2 Bass kernel for nn_Dilation2D (morphological dilation, max-plus conv).

    out[b,y,x,c] = max_{i,j} ( x_pad[b, y+i-1, x+j-1, c] + w[i,j,c] )

Sharding: pure data parallel over batch B=8 -> one image per NeuronCore.

Per-core layout: partitions p = hb*32 + c  (hb = one of 4 row-blocks of 128
image rows, c = channel).  Free dim = (row, x).  In this layout a tap is
    acc = max(x_tile[row+i, x+j] + w_vec[p], acc)
with w a per-partition scalar, so the adds are tensor_scalar/ACT-bias ops
and the maxes are 16-bit tensor_tensor ops in the DVE 2x perf mode.

The (y,x,c) -> ((hb,c),(y,x)) relayout rides the otherwise-idle TensorE:
big per-hb loads land [x-pos -> partitions, (xc,hb,c) -> free] (128B-
contiguous DRAM reads), PE transpose-mode matmuls flip each [128,128] tile
into PSUM, and the ACT PSUM->SBUF copy doubles as the f32->fp16 cast.
Reverse on the way out (fp16 transposes, ACT copy casts back to f32).
Tap work is split across DVE (tensor_scalar adds + all maxes) and ACT
(bias-adds).
"""

import numpy as np

import concourse.bass as bass
import concourse.bacc as bacc
import concourse.tile as tile
import concourse.dve_ops as dve_ops
from concourse import mybir
from concourse.bass_utils import run_bass_kernel_spmd
from concourse.dve_spec import Spec, Src0, Src1, C0, maxx, lower
from concourse.dve_uop import (
    DveOpSpec, UopConfig, UopDpConfig, InpSel, OutSel, OutPath, AluOp,
    AluInp, DelayInp, Trigger,
)

# Problem constants (hardcoded per contract).
B, H, W, C = 8, 512, 512, 32
KH, KW = 4, 4
HBLK = 4               # row blocks on partitions
HB = H // HBLK         # 128 rows per block
XC = W // 128          # 4 x-chunks of 128 pixels
YT = 8                 # output rows per chunk
NCHUNK = HB // YT      # 16 chunks
RT = YT + KH - 1       # 11 input rows per chunk (with halo)
XOFF = 16              # x=0 lives at column 16 (32B alignment for the xbar)
XW = 544               # padded row width: [15]=x=-1, [16,528)=x, [528,530)=halo
NEG = -60000.0         # -inf stand-in that fits fp16

F32 = mybir.dt.float32
F16 = mybir.dt.float16
AX = mybir.AluOpType

# tap routing. GPSIMD supports no float tensor ops at the ISA level on this
# toolchain, so all 16 taps run on DVE as fused custom ops ((x+w) max acc,
# hand-written 2x_1p uop program; first tap is a tensor_scalar add). The 2x
# mode needs 4B-aligned windows: odd-j taps read the x-tile directly; even-j
# taps read xt_odd, a one-element-left-shifted copy (made by ACT, which is
# alignment-blind), where their windows land on even offsets.
DVE_TAPS = [(i, j) for i in range(KH) for j in range(KW) if j % 2 == 1]
EVEN_TAPS = [(i, j) for i in range(KH) for j in range(KW) if j % 2 == 0]

_ENABLE, _DISABLE = 1, 0


def _dp_stage(op, a, b, cap_lanes=(), pass_lanes=()):
    delay = [DelayInp.PREV_ALU_OUT] * 7
    enable = [_DISABLE] * 7
    for ln in pass_lanes:
        delay[ln] = DelayInp.PREV_DELAY
        enable[ln] = _ENABLE
    for ln in cap_lanes:
        delay[ln] = DelayInp.PREV_ALU_OUT
        enable[ln] = _ENABLE
    return UopDpConfig(
        op=op, alu_src0=a, alu_src1=b,
        delay=delay, alu_out_enable=_ENABLE, swap_enable=_DISABLE,
        alu_out_a_enable=_DISABLE, alu_out_b_enable=_DISABLE,
        delay_enable=enable, idx0_sel=0, idx1_sel=0,
    )


def _build_2x_uop():
    """2x_1p program for out = max(src0 + c0, src1): lo pair on slices 0/2,
    hi pair on slices 1/3, packed write via WR0_LO/WR0_HI."""
    inp = [InpSel.ZERO] * 8
    inp_en = [_DISABLE] * 8
    for ln, sel in {0: InpSel.SRC_0, 1: InpSel.CONST_0, 2: InpSel.SRC_1,
                    3: InpSel.SRC_0_HI, 4: InpSel.SRC_1_HI}.items():
        inp[ln + 1] = sel
        inp_en[ln + 1] = _ENABLE
    D = AluInp
    dp = [
        _dp_stage(AluOp.ADD, D.PREV_DELAY_0, D.PREV_DELAY_1,
                  pass_lanes=(0, 1, 2, 3, 4)),
        _dp_stage(AluOp.ADD, D.PREV_DELAY_3, D.PREV_DELAY_1,
                  cap_lanes=(5,), pass_lanes=(1, 2, 4)),
        _dp_stage(AluOp.MAX, D.PREV_DELAY_5, D.PREV_DELAY_2,
                  cap_lanes=(0,), pass_lanes=(4,)),
        _dp_stage(AluOp.MAX, D.PREV_DELAY_0, D.PREV_DELAY_4,
                  cap_lanes=(1,)),
        _dp_stage(AluOp.BYPASS, D.PREV_ALU_OUT, D.PREV_ALU_OUT,
                  pass_lanes=(1,)),
        _dp_stage(AluOp.BYPASS, D.PREV_ALU_OUT, D.PREV_ALU_OUT,
                  pass_lanes=(1,)),
        _dp_stage(AluOp.BYPASS, D.PREV_ALU_OUT, D.PREV_ALU_OUT,
                  pass_lanes=(1,)),
        _dp_stage(AluOp.BYPASS, D.PREV_ALU_OUT, D.PREV_ALU_OUT,
                  pass_lanes=(1,)),
    ]
    out = {OutPath.WR0_LO: OutSel.DELAY_1, OutPath.WR0_HI: OutSel.ALU_OUT,
           OutPath.WR1_LO: OutSel.ALU_OUT, OutPath.WR1_HI: OutSel.ALU_OUT}
    out_en = {OutPath.WR0_LO: _ENABLE, OutPath.WR0_HI: _ENABLE,
              OutPath.WR1_LO: _DISABLE, OutPath.WR1_HI: _DISABLE}
    return UopConfig(
        datapath_config=dp, inp=inp, inp_enable=inp_en,
        out=out, out_enable=out_en, require_inp0=1, require_inp1=1,
        trigger=(Trigger.SRC_TENSOR_DONE, Trigger.NONE, Trigger.NONE),
        next_uop=(0, 0, 0), repeat_count=0,
    )


def _register_tap_op():
    """Register TAP_MAXADD_ANT (out = max(in0 + s0, in1)) with a base 1x
    program from lower() and the hand-written 2x_1p slot."""
    name = "TAP_MAXADD_ANT"
    for o in dve_ops.OPS:
        if o.name == name:
            return o
    def _ref(in0, in1, s0, s1, imm2):
        s0 = np.asarray(s0)
        if s0.ndim and s0.ndim < in0.ndim:
            s0 = s0.reshape(s0.shape[0], *([1] * (in0.ndim - 1)))
        in1 = np.asarray(in1).reshape(in0.shape)
        return np.maximum(in0.astype(np.float32) + s0, in1)

    spec = Spec(body=maxx(Src0 + C0, Src1), reference=_ref)
    row = dve_ops._CUSTOM_DVE_ROW_BASE + len(dve_ops.OPS)
    u2 = _build_2x_uop()
    u2.validate("v3")
    full = DveOpSpec(name=name, opcode=row, uops=lower(spec, ver="v3"),
                     uops_2x=[u2], rd1_en=True, perf_max=1)
    full.validate("v3")
    op = dve_ops.DveOp(name, spec, subdim=False,
                       uops_sha={"v3": full.sha("v3")})
    dve_ops.OPS.append(op)
    dve_ops._SUB_OPCODE_FOR_NAME[name] = row
    dve_ops.CUSTOM_DVE_SPECS[name] = spec
    dve_ops._COMPILE_CACHE[(name, "v3")] = full
    return op


def _build_program(repeat: int = 1):
    tap_op = _register_tap_op()
    nc = bacc.Bacc("TRN2", target_bir_lowering=False, debug=False)
    x_d = nc.dram_tensor("x", [H, W, C], F32, kind="ExternalInput").ap()
    w_d = nc.dram_tensor("w", [KH, KW, C], F32, kind="ExternalInput").ap()
    o_d = nc.dram_tensor("out", [H, W, C], F32, kind="ExternalOutput").ap()

    # DRAM views: q = x-position within an x-chunk; dims [q, y, xc, hb, c]
    x_v = x_d.rearrange("(hb y) (xc q) c -> q y xc hb c", hb=HBLK, q=128)
    o_v = o_d.rearrange("(hb y) (xc q) c -> q y xc hb c", hb=HBLK, q=128)

    with tile.TileContext(nc) as tc:
        consts = tc.alloc_tile_pool(name="consts", bufs=1)
        w_sb = consts.tile([128, KH * KW], F32)
        w_r = w_d.rearrange("i j c -> c (i j)")
        for hb in range(HBLK):
            nc.sync.dma_start(out=w_sb[32 * hb : 32 * (hb + 1), :], in_=w_r)
        import concourse.masks as masks
        id32 = consts.tile([128, 128], F32)
        masks.make_identity(nc, id32[:])
        id16 = consts.tile([128, 128], F16)
        masks.make_identity(nc, id16[:])

        pre32_pool = tc.alloc_tile_pool(name="pre32", bufs=2)
        xbuf_pool = tc.alloc_tile_pool(name="xbuf", bufs=2)
        acc_pool = tc.alloc_tile_pool(name="acc", bufs=2)
        ost_pool = tc.alloc_tile_pool(name="ost", bufs=2)
        psi_pool = tc.alloc_tile_pool(name="psi", bufs=3, space="PSUM")
        pso_pool = tc.alloc_tile_pool(name="pso", bufs=3, space="PSUM")

        for ck_rep in range(NCHUNK * repeat):
            ck = ck_rep % NCHUNK
            y0 = ck * YT  # first output row (within each hb block)

            # ---- load: DRAM -> pre32 [q, (r, xc, hb, c)], one big DMA per
            # hb ((y,xc) and (r,xc) merge, keeping APs at 3 dims) ----
            r_lo = 1 if ck == 0 else 0
            r_hi = RT - 2 if ck == NCHUNK - 1 else RT
            pre32 = pre32_pool.tile([128, RT * XC * HBLK * C], F32)
            p32v = pre32[:].rearrange(
                "q (r xc hb c) -> q r xc hb c", r=RT, xc=XC, hb=HBLK, c=C
            )
            for hb in range(HBLK):
                nc.sync.dma_start(
                    out=p32v[:, r_lo:r_hi, :, hb],
                    in_=x_v[:, y0 - 1 + r_lo : y0 - 1 + r_hi, :, hb],
                )
                if ck == 0:
                    # r=0 is y_loc=-1: row 127 of block hb-1 (hb=0 gets a
                    # dummy row; masked to NEG after the relayout)
                    nc.sync.dma_start(
                        out=p32v[:, 0, :, hb],
                        in_=x_v[:, HB - 1 if hb else 0, :, max(hb - 1, 0)],
                    )
                if ck == NCHUNK - 1:
                    # r in {RT-2, RT-1} are y_loc {128,129}: rows 0,1 of hb+1
                    # (hb=3 gets dummy rows; masked to NEG after)
                    nc.sync.dma_start(
                        out=p32v[:, RT - 2 : RT, :, hb],
                        in_=x_v[:, 0:2, :, min(hb + 1, HBLK - 1)],
                    )

            # ---- relayout: PE transpose-mode matmuls [q,(hb,c)]->[(hb,c),q]
            # into PSUM; ACT copies PSUM -> x-tile, casting f32 -> fp16 ----
            xt = xbuf_pool.tile([128, RT * XW], F16)
            xt_v = xt[:].rearrange("p (r x) -> p r x", r=RT, x=XW)
            for r in range(RT):
                ps = psi_pool.tile([128, XC * 128], F32)  # one PSUM bank
                for xc in range(XC):
                    nc.tensor.matmul(
                        ps[:, 128 * xc : 128 * (xc + 1)],
                        p32v[:, r, xc],
                        id32[:],
                        start=(xc == 0),
                        stop=(xc == XC - 1),
                        is_transpose=True,
                        skip_group_check=True,
                    )
                nc.scalar.copy(xt_v[:, r, XOFF : XOFF + W], ps[:])
            # borders: left halo col (x=-1), right cols [528, 544)
            nc.gpsimd.memset(xt_v[:, :, XOFF - 1 : XOFF], NEG)
            nc.gpsimd.memset(xt_v[:, :, XOFF + W :], NEG)
            if ck == 0:
                nc.gpsimd.memset(xt_v[0:32, 0, :], NEG)          # hb=0, y=-1
            if ck == NCHUNK - 1:
                nc.gpsimd.memset(xt_v[96:128, RT - 2 : RT, :], NEG)  # hb=3

            # one-element-left-shifted copy: even-j windows land 4B-aligned
            xto = xbuf_pool.tile([128, RT * XW], F16, tag="xt_odd")
            xto_v = xto[:].rearrange("p (r x) -> p r x", r=RT, x=XW)
            nc.scalar.copy(
                xto_v[:, :, XOFF - 2 : XOFF - 2 + W + 2],
                xt_v[:, :, XOFF - 1 : XOFF - 1 + W + 2],
            )

            def win(i, j):
                if j % 2 == 1:
                    return xt_v[:, i : i + YT, XOFF - 1 + j : XOFF - 1 + j + W]
                return xto_v[:, i : i + YT, XOFF - 2 + j : XOFF - 2 + j + W]

            def w_ap(i, j):
                t = i * KW + j
                return w_sb[:, t : t + 1]

            # ---- taps ----
            acc = acc_pool.tile([128, YT * W], F16, tag="acc_dve")
            acc_v = acc[:].rearrange("p (r x) -> p r x", r=YT, x=W)

            # DVE chain: first tap straight into acc (tensor_scalar add),
            # all remaining taps as one fused custom op each.
            (i0, j0) = DVE_TAPS[0]
            nc.vector.tensor_scalar(
                acc_v[:], win(i0, j0), w_ap(i0, j0), None, AX.add
            )
            for (i, j) in DVE_TAPS[1:] + EVEN_TAPS:
                nc.vector._custom_dve(
                    tap_op, out=acc_v[:], in0=win(i, j), in1=acc_v[:],
                    s0=w_ap(i, j),
                )

            # ---- transpose back on PE (fp16), ACT copy casts to f32, store
            ost32 = ost_pool.tile([128, YT * XC * HBLK * C], F32, tag="o32")
            o32v = ost32[:].rearrange(
                "q (r xc hb c) -> q r xc hb c", r=YT, xc=XC, hb=HBLK, c=C
            )
            for r in range(YT):
                ps = pso_pool.tile([128, XC * 128], F16)  # half a PSUM bank
                for xc in range(XC):
                    nc.tensor.matmul(
                        ps[:, 128 * xc : 128 * (xc + 1)],
                        acc_v[:, r, 128 * xc : 128 * (xc + 1)],
                        id16[:],
                        start=(xc == 0),
                        stop=(xc == XC - 1),
                        is_transpose=True,
                        skip_group_check=True,
                    )
                nc.scalar.copy(o32v[:, r], ps[:])
            for hb in range(HBLK):
                nc.sync.dma_start(
                    out=o_v[:, y0 : y0 + YT, :, hb], in_=o32v[:, :, :, hb]
                )

        for p in (pso_pool, psi_pool, ost_pool, acc_pool,
                  xbuf_pool, pre32_pool, consts):
            p.release()

    # Enable the 2x perf slot on the real instructions (byte-36[7:6]).
    # Setting it on the BassInstruction wrapper returned by _custom_dve
    # does NOT reach the underlying instruction.
    for bb in nc.m.functions[0].blocks:
        for i in bb.instructions:
            if type(i).__name__ == "InstCustomDveAnt":
                i.perf_max = 1
    nc.compile()
    return nc


_CACHED = {}


def _get_program(repeat: int = 1):
    if repeat not in _CACHED:
        _CACHED[repeat] = _build_program(repeat)
    return _CACHED[repeat]


def kernel(x: np.ndarray, w: np.ndarray, _trace: bool = False,
           _repeat: int = 1):
    """Full inputs in, full output out. Shards batch across 8 cores."""
    x = np.ascontiguousarray(np.asarray(x), dtype=np.float32)
    w = np.ascontiguousarray(np.asarray(w), dtype=np.float32)
    assert x.shape == (B, H, W, C) and w.shape == (KH, KW, C)
    nc = _get_program(_repeat)
    core_ids = list(range(B))
    in_maps = [{"x": x[b], "w": w} for b in range(B)]
    res = run_bass_kernel_spmd(nc, in_maps, core_ids, trace=_trace)
    out = np.stack([res.results[i]["out"] for i in range(B)], axis=0)
    if _trace:
        kernel.last_exec_time_ns = res.exec_time_ns
        kernel.last_results = res
    return out


if __name__ == "__main__":
    rng = np.random.default_rng(0)
    x = rng.standard_normal((B, H, W, C), dtype=np.float32)
    w = (rng.standard_normal((KH, KW, C)) * 0.1).astype(np.float32)
    out = kernel(x, w)
    print("out", out.shape, out.dtype, float(out.mean()))



# revision 12
# speedup vs baseline: 3.4096x; 3.4096x over previous
"""Trainium2 Bass kernel for nn_Dilation2D (morphological dilation, max-plus conv).

    out[b,y,x,c] = max_{i,j} ( x_pad[b, y+i-1, x+j-1, c] + w[i,j,c] )

Sharding: pure data parallel over batch B=8 -> one image per NeuronCore.

Per-core layout: partitions p = hb*32 + c  (hb = one of 4 row-blocks of 128
image rows, c = channel).  Free dim = (row, x).  In this layout a tap is
    acc = max(x_tile[row+i, x+j] + w_vec[p], acc)
with w a per-partition scalar (CONST lanes), so taps fuse on the DVE.

Two structural wins over the previous version:

1. Fused PAIR-TAP custom DVE ops: one 2x_1P uop program computes TWO taps
   plus the acc max in a single pass,
       out[t] = max(in0[t-2] + cA, in0[t] + cA + cB', acc[t])
   using temporal CURR_ALU_OUT delay captures for the in0[t-2] history
   (exactly 8 ALU slices: 4 ADD + 4 MAX).  16 taps -> 8 DVE passes.
   Streams are flat [p, 8*544] full rows; the 2-element-late garbage at
   each row start lands in the 16-column left margin.

2. 512-byte DMA descriptors both directions: DRAM<->SBUF staging uses
   partitions q = x//4 so each descriptor moves (x4,c) = 4*32*4B = 512B
   of contiguous DRAM (vs 128B before, which paid the sub-512B RMW 2x
   penalty).  PE transpose tiles use f-slices (hb,c) per (r, x4); the
   PSUM->SBUF ACT copy interleaves x = 4q+x4 back into sequential x.

The (y,x,c) relayout rides the otherwise-idle TensorE (transpose-mode
matmuls), ACT does PSUM->SBUF copies (with f32<->fp16 casts), and the
one-element-shifted xto copy (for even-j tap alignment) runs on GpSimd.
"""

import numpy as np

import concourse.bass as bass
import concourse.bacc as bacc
import concourse.tile as tile
import concourse.dve_ops as dve_ops
from concourse import mybir
from concourse.bass_utils import run_bass_kernel_spmd
from concourse.dve_spec import Spec, Src0, Src1, C0, C1, maxx
from concourse.dve_uop import (
    DveOpSpec, UopConfig, UopDpConfig, InpSel, OutSel, OutPath, AluOp,
    AluInp, DelayInp, Trigger,
)

# Problem constants (hardcoded per contract).
B, H, W, C = 8, 512, 512, 32
KH, KW = 4, 4
HBLK = 4               # row blocks on partitions
HB = H // HBLK         # 128 rows per block
X4 = 4                 # x-positions per DMA descriptor (512B runs)
YT = 8                 # output rows per chunk
NCHUNK = HB // YT      # 16 chunks
RT = YT + KH - 1       # 11 input rows per chunk (with halo)
XOFF = 16              # x=0 lives at column 16
XW = 544               # padded row width: [15]=x=-1, [16,528)=x, [528,530)=halo
FLAT = YT * XW         # flat acc stream length (4352, even)
XTN = RT * XW + 8      # xt/xto alloc incl. tail pad read by the i=3 window
NEG = -60000.0         # -inf stand-in that fits fp16

F32 = mybir.dt.float32
F16 = mybir.dt.float16

_ENABLE, _DISABLE = 1, 0
_A = AluInp
_D = DelayInp


def _stage(op, a, b=None, cap=None, passes=()):
    """One datapath block: ALU = op(a, b); delay lanes in `passes` forward
    the upstream value; lanes in `cap` capture from the given DelayInp
    source (PREV_ALU_OUT = upstream flop, CURR_ALU_OUT = this slice's own
    flop from the previous element -> a one-element temporal delay)."""
    delay = [_D.PREV_ALU_OUT] * 7
    enable = [_DISABLE] * 7
    for ln in passes:
        delay[ln] = _D.PREV_DELAY
        enable[ln] = _ENABLE
    for ln, src in (cap or {}).items():
        delay[ln] = src
        enable[ln] = _ENABLE
    return UopDpConfig(
        op=op, alu_src0=a, alu_src1=b if b is not None else a,
        delay=delay, alu_out_enable=_ENABLE, swap_enable=_DISABLE,
        alu_out_a_enable=_DISABLE, alu_out_b_enable=_DISABLE,
        delay_enable=enable, idx0_sel=0, idx1_sel=0,
    )


def _uop(inp_map, dp, out_map):
    """Assemble a UopConfig. inp_map: {lane: InpSel}; out_map:
    {OutPath: OutSel} for enabled write paths."""
    inp = [InpSel.ZERO] * 8
    inp_en = [_DISABLE] * 8
    for ln, sel in inp_map.items():
        inp[ln] = sel
        inp_en[ln] = _ENABLE
    out = {p: OutSel.ALU_OUT for p in OutPath}
    out_en = {p: _DISABLE for p in OutPath}
    for p, sel in out_map.items():
        out[p] = sel
        out_en[p] = _ENABLE
    has_src1 = any(
        s in (InpSel.SRC_1, InpSel.SRC_1_HI) for s in inp_map.values()
    )
    return UopConfig(
        datapath_config=dp, inp=inp, inp_enable=inp_en,
        out=out, out_enable=out_en,
        require_inp0=1, require_inp1=1 if has_src1 else 0,
        trigger=(Trigger.SRC_TENSOR_DONE, Trigger.NONE, Trigger.NONE),
        next_uop=(0, 0, 0), repeat_count=0,
    )


def _acc_2x():
    """2x_1P: out pair = max(wA(prev pair), wB(this pair), acc pair)
    with wA = in0 + c0, wB = wA + c1.  4 ADD + 4 MAX = all 8 slices."""
    inp = {0: InpSel.SRC_0, 1: InpSel.SRC_0_HI, 2: InpSel.SRC_1,
           3: InpSel.SRC_1_HI, 4: InpSel.CONST_0, 5: InpSel.CONST_1}
    # lanes at stage0: PD0=hi PD1=accL PD2=accH PD3=cA PD4=cB'
    dp = [
        # flop=wA_lo(p); D5<-wA_lo(p-1)
        _stage(AluOp.ADD, _A.PREV_ALU_OUT, _A.PREV_DELAY_3,
               cap={5: _D.CURR_ALU_OUT}, passes=(0, 1, 2, 3, 4)),
        # flop=wA_hi(p); D0<-wA_hi(p-1); D3<-wA_lo(p)
        _stage(AluOp.ADD, _A.PREV_DELAY_0, _A.PREV_DELAY_3,
               cap={0: _D.CURR_ALU_OUT, 3: _D.PREV_ALU_OUT},
               passes=(1, 2, 4, 5)),
        # flop=wB_lo(p); D3<-wA_hi(p)
        _stage(AluOp.ADD, _A.PREV_DELAY_3, _A.PREV_DELAY_4,
               cap={3: _D.PREV_ALU_OUT}, passes=(0, 1, 2, 4, 5)),
        # flop=wB_hi(p); D3<-wB_lo(p)
        _stage(AluOp.ADD, _A.PREV_DELAY_3, _A.PREV_DELAY_4,
               cap={3: _D.PREV_ALU_OUT}, passes=(0, 1, 2, 5)),
        # flop=m_lo=max(wA_lo(p-1), wB_lo(p)); D3<-wB_hi(p)
        _stage(AluOp.MAX, _A.PREV_DELAY_5, _A.PREV_DELAY_3,
               cap={3: _D.PREV_ALU_OUT}, passes=(0, 1, 2)),
        # flop=m_hi=max(wA_hi(p-1), wB_hi(p)); D0<-m_lo
        _stage(AluOp.MAX, _A.PREV_DELAY_0, _A.PREV_DELAY_3,
               cap={0: _D.PREV_ALU_OUT}, passes=(1, 2)),
        # flop=out_lo=max(m_lo, accL); D0<-m_hi
        _stage(AluOp.MAX, _A.PREV_DELAY_0, _A.PREV_DELAY_1,
               cap={0: _D.PREV_ALU_OUT}, passes=(2,)),
        # flop=out_hi=max(m_hi, accH); D1<-out_lo
        _stage(AluOp.MAX, _A.PREV_DELAY_0, _A.PREV_DELAY_2,
               cap={1: _D.PREV_ALU_OUT}),
    ]
    return _uop(inp, dp, {OutPath.WR0_LO: OutSel.DELAY_1,
                          OutPath.WR0_HI: OutSel.ALU_OUT})


def _acc_1x():
    """1x: out[t] = max(wA(t-2), wB(t), acc[t])."""
    inp = {0: InpSel.SRC_0, 1: InpSel.SRC_1, 2: InpSel.CONST_0,
           3: InpSel.CONST_1}
    # lanes: PD0=acc PD1=cA PD2=cB'
    dp = [
        # flop=wA(t); D5<-wA(t-1)
        _stage(AluOp.ADD, _A.PREV_ALU_OUT, _A.PREV_DELAY_1,
               cap={5: _D.CURR_ALU_OUT}, passes=(0, 2)),
        # flop=wA(t-1); D5<-wA(t-2); D3<-wA(t)
        _stage(AluOp.BYPASS, _A.PREV_DELAY_5,
               cap={5: _D.CURR_ALU_OUT, 3: _D.PREV_ALU_OUT}, passes=(0, 2)),
        # flop=wB(t)
        _stage(AluOp.ADD, _A.PREV_DELAY_3, _A.PREV_DELAY_2, passes=(0, 5)),
        _stage(AluOp.MAX, _A.PREV_ALU_OUT, _A.PREV_DELAY_5, passes=(0,)),
        _stage(AluOp.MAX, _A.PREV_ALU_OUT, _A.PREV_DELAY_0),
        _stage(AluOp.BYPASS, _A.PREV_ALU_OUT),
        _stage(AluOp.BYPASS, _A.PREV_ALU_OUT),
        _stage(AluOp.BYPASS, _A.PREV_ALU_OUT),
    ]
    return _uop(inp, dp, {OutPath.WR0_LO: OutSel.ALU_OUT})


def _init_2x():
    """2x_1P 'init' variant: identical to _acc_2x but the final two MAXes
    are BYPASSes -- in1 is READ (keeping the op two-source so the RTL mode
    detector stays at 2x_1P; a single-src 16-bit aligned op would select
    the unpopulated 2x_2P/4x slots and crash the engine) but ignored."""
    inp = {0: InpSel.SRC_0, 1: InpSel.SRC_0_HI, 2: InpSel.SRC_1,
           3: InpSel.SRC_1_HI, 4: InpSel.CONST_0, 5: InpSel.CONST_1}
    dp = [
        _stage(AluOp.ADD, _A.PREV_ALU_OUT, _A.PREV_DELAY_3,
               cap={5: _D.CURR_ALU_OUT}, passes=(0, 1, 2, 3, 4)),
        _stage(AluOp.ADD, _A.PREV_DELAY_0, _A.PREV_DELAY_3,
               cap={0: _D.CURR_ALU_OUT, 3: _D.PREV_ALU_OUT},
               passes=(1, 2, 4, 5)),
        _stage(AluOp.ADD, _A.PREV_DELAY_3, _A.PREV_DELAY_4,
               cap={3: _D.PREV_ALU_OUT}, passes=(0, 1, 2, 4, 5)),
        _stage(AluOp.ADD, _A.PREV_DELAY_3, _A.PREV_DELAY_4,
               cap={3: _D.PREV_ALU_OUT}, passes=(0, 1, 2, 5)),
        _stage(AluOp.MAX, _A.PREV_DELAY_5, _A.PREV_DELAY_3,
               cap={3: _D.PREV_ALU_OUT}, passes=(0, 1, 2)),
        _stage(AluOp.MAX, _A.PREV_DELAY_0, _A.PREV_DELAY_3,
               cap={0: _D.PREV_ALU_OUT}, passes=(1, 2)),
        _stage(AluOp.BYPASS, _A.PREV_DELAY_0,
               cap={0: _D.PREV_ALU_OUT}, passes=(2,)),
        _stage(AluOp.BYPASS, _A.PREV_DELAY_0,
               cap={1: _D.PREV_ALU_OUT}),
    ]
    return _uop(inp, dp, {OutPath.WR0_LO: OutSel.DELAY_1,
                          OutPath.WR0_HI: OutSel.ALU_OUT})


def _init_1x():
    inp = {0: InpSel.SRC_0, 1: InpSel.SRC_1, 2: InpSel.CONST_0,
           3: InpSel.CONST_1}
    # lanes: PD0=acc(ignored) PD1=cA PD2=cB'
    dp = [
        _stage(AluOp.ADD, _A.PREV_ALU_OUT, _A.PREV_DELAY_1,
               cap={5: _D.CURR_ALU_OUT}, passes=(2,)),
        _stage(AluOp.BYPASS, _A.PREV_DELAY_5,
               cap={5: _D.CURR_ALU_OUT, 3: _D.PREV_ALU_OUT}, passes=(2,)),
        _stage(AluOp.ADD, _A.PREV_DELAY_3, _A.PREV_DELAY_2, passes=(5,)),
        _stage(AluOp.MAX, _A.PREV_ALU_OUT, _A.PREV_DELAY_5),
        _stage(AluOp.BYPASS, _A.PREV_ALU_OUT),
        _stage(AluOp.BYPASS, _A.PREV_ALU_OUT),
        _stage(AluOp.BYPASS, _A.PREV_ALU_OUT),
        _stage(AluOp.BYPASS, _A.PREV_ALU_OUT),
    ]
    return _uop(inp, dp, {OutPath.WR0_LO: OutSel.ALU_OUT})


def _sliding_ref(in0, s0, s1):
    in0 = np.asarray(in0, np.float32)
    a = in0 + s0
    b = a + s1
    ash = np.concatenate([np.full_like(a[..., :2], NEG), a[..., :-2]], -1)
    return np.maximum(ash, b)


def _register_op(name, body, ref, u1x, u2x, rd1):
    for o in dve_ops.OPS:
        if o.name == name:
            return o
    spec = Spec(body=body, reference=ref)
    row = dve_ops._CUSTOM_DVE_ROW_BASE + len(dve_ops.OPS)
    u2x.validate("v3")
    full = DveOpSpec(name=name, opcode=row, uops=[u1x], uops_2x=[u2x],
                     rd1_en=rd1, perf_max=1)
    full.validate("v3")
    op = dve_ops.DveOp(name, spec, subdim=False,
                       uops_sha={"v3": full.sha("v3")})
    dve_ops.OPS.append(op)
    dve_ops._SUB_OPCODE_FOR_NAME[name] = row
    dve_ops.CUSTOM_DVE_SPECS[name] = spec
    dve_ops._COMPILE_CACHE[(name, "v3")] = full
    return op


def _register_tap_ops():
    def _ref_acc(in0, in1, s0, s1, imm2):
        r = _sliding_ref(in0, s0, s1)
        return np.maximum(r, np.asarray(in1, np.float32).reshape(r.shape))

    def _ref_init(in0, in1, s0, s1, imm2):
        return _sliding_ref(in0, s0, s1)

    acc_op = _register_op(
        "PAIRTAP_ACC_ANT", maxx(Src0 + C0, Src1), _ref_acc,
        _acc_1x(), _acc_2x(), rd1=True,
    )
    init_op = _register_op(
        "PAIRTAP_INIT_ANT", maxx(Src0 + C0, Src1), _ref_init,
        _init_1x(), _init_2x(), rd1=True,
    )
    return acc_op, init_op


def _build_program(repeat: int = 1):
    acc_op, init_op = _register_tap_ops()
    nc = bacc.Bacc("TRN2", target_bir_lowering=False, debug=False)
    x_d = nc.dram_tensor("x", [H, W, C], F32, kind="ExternalInput").ap()
    w_d = nc.dram_tensor("w", [KH, KW, C], F32, kind="ExternalInput").ap()
    o_d = nc.dram_tensor("out", [H, W, C], F32, kind="ExternalOutput").ap()

    # Input view: q = x within a 128-wide block (baseline layout), so the
    # per-(r,xc) transpose f-slice (hb,c) is contiguous.  The f32->fp16
    # cast rides the SWDGE DMA (gpsimd), halving SBUF-side bytes.
    x_v = x_d.rearrange("(hb y) (xc q) c -> q y xc hb c", hb=HBLK, q=128)
    # Output view: q = x//4 on partitions -> DRAM descriptors of
    # (x4,c) = 512B, fed by stride-4 PE transposes of acc.
    o_r = o_d.rearrange("(hb y) (q xf) c -> q y hb (xf c)", hb=HBLK, xf=X4)

    with tile.TileContext(nc) as tc:
        consts = tc.alloc_tile_pool(name="consts", bufs=1)
        w_sb = consts.tile([128, KH * KW], F32)
        w_r = w_d.rearrange("i j c -> c (i j)")
        for hb in range(HBLK):
            nc.sync.dma_start(out=w_sb[32 * hb : 32 * (hb + 1), :], in_=w_r)
        # cB' = w[i][jp+2] - w[i][jp] per (i, parity jp): col i*2+jp.
        w_cb = consts.tile([128, KH * 2], F32)
        wv = w_sb[:].rearrange("p (i t) -> p i t", i=KH, t=KW)
        cbv = w_cb[:].rearrange("p (i jp) -> p i jp", i=KH, jp=2)
        nc.vector.tensor_sub(cbv, wv[:, :, 2:4], wv[:, :, 0:2])
        import concourse.masks as masks
        id32 = consts.tile([128, 128], F32)
        masks.make_identity(nc, id32[:])
        id16 = consts.tile([128, 128], F16)
        masks.make_identity(nc, id16[:])

        pre32_pool = tc.alloc_tile_pool(name="pre32", bufs=2)
        xbuf_pool = tc.alloc_tile_pool(name="xbuf", bufs=2)
        acc_pool = tc.alloc_tile_pool(name="acc", bufs=2)
        ost_pool = tc.alloc_tile_pool(name="ost", bufs=2)
        psi_pool = tc.alloc_tile_pool(name="psi", bufs=3, space="PSUM")
        pso_pool = tc.alloc_tile_pool(name="pso", bufs=3, space="PSUM")

        xt_prev = xto_prev = None
        for ck_rep in range(NCHUNK * repeat):
            ck = ck_rep % NCHUNK
            y0 = ck * YT  # first output row (within each hb block)

            # ---- load: DRAM f32 -> pre16 fp16 [q, (r, xc, hb, c)] via
            # SWDGE cast DMA (one per hb).  Rows 0..2 (the 3-row halo) are
            # carried over from the previous chunk's xt/xto instead of
            # being re-read, except at ck==0.
            r_lo = 1 if ck == 0 else 3
            r_hi = RT - 2 if ck == NCHUNK - 1 else RT
            XC = W // 128
            pre16 = pre32_pool.tile([128, RT * XC * HBLK * C], F16)
            p16v = pre16[:].rearrange(
                "q (r xc hb c) -> q r xc hb c", r=RT, xc=XC, hb=HBLK, c=C
            )
            for hb in range(HBLK):
                nc.gpsimd.dma_start(
                    out=p16v[:, r_lo:r_hi, :, hb],
                    in_=x_v[:, y0 - 1 + r_lo : y0 - 1 + r_hi, :, hb],
                )
                if ck == 0:
                    # r=0 is y_loc=-1: row 127 of block hb-1 (hb=0 gets a
                    # dummy row; masked to NEG after the relayout)
                    nc.gpsimd.dma_start(
                        out=p16v[:, 0, :, hb],
                        in_=x_v[:, HB - 1 if hb else 0, :, max(hb - 1, 0)],
                    )
                if ck == NCHUNK - 1:
                    # r in {RT-2, RT-1} are y_loc {128,129}: rows 0,1 of hb+1
                    nc.gpsimd.dma_start(
                        out=p16v[:, RT - 2 : RT, :, hb],
                        in_=x_v[:, 0:2, :, min(hb + 1, HBLK - 1)],
                    )

            # ---- relayout fresh rows: per (r, xc) fp16 PE transpose of
            # [q, (hb c)] tiles; ACT copies PSUM -> xt rows
            t_lo = 0 if ck == 0 else 3
            xt = xbuf_pool.tile([128, XTN], F16, tag="xt")
            xto = xbuf_pool.tile([128, XTN], F16, tag="xto")
            if ck != 0:
                # halo rows 0..2 <- previous chunk's rows 8..10 (incl. NEG
                # margins).  xt via SBUF->SBUF DMA, xto via GpSimd copy --
                # ACT and DVE are the loaded engines.
                nc.sync.dma_start(
                    out=xt[:, 0 : 3 * XW], in_=xt_prev[:, 8 * XW : RT * XW]
                )
                nc.gpsimd.tensor_copy(
                    xto[:, 0 : 3 * XW], xto_prev[:, 8 * XW : RT * XW]
                )
            for r in range(t_lo, RT):
                ps = psi_pool.tile([128, XC * 128], F16)  # half a PSUM bank
                for xc in range(XC):
                    nc.tensor.matmul(
                        ps[:, 128 * xc : 128 * (xc + 1)],
                        p16v[:, r, xc],
                        id16[:],
                        start=(xc == 0),
                        stop=(xc == XC - 1),
                        is_transpose=True,
                        skip_group_check=True,
                    )
                nc.scalar.copy(xt[:, r * XW + XOFF : r * XW + XOFF + W], ps[:])

            # borders on fresh rows: left halo col (x=-1), right [528, 544)
            xt_v = xt[:, : RT * XW].rearrange("p (r x) -> p r x", r=RT, x=XW)
            nc.gpsimd.memset(xt_v[:, t_lo:, XOFF - 1 : XOFF], NEG)
            nc.gpsimd.memset(xt_v[:, t_lo:, XOFF + W :], NEG)
            if ck == 0:
                nc.gpsimd.memset(xt_v[0:32, 0, :], NEG)          # hb=0, y=-1
            if ck == NCHUNK - 1:
                nc.gpsimd.memset(xt_v[96:128, RT - 2 : RT, :], NEG)  # hb=3

            # one-element-left-shifted copy of the fresh rows for even-j
            # tap pairs (ACT is alignment-blind)
            nc.scalar.copy(
                xto[:, t_lo * XW : RT * XW + 4],
                xt[:, t_lo * XW + 1 : RT * XW + 5],
            )
            xt_prev, xto_prev = xt, xto

            # ---- taps: 8 fused pair ops, odd-j pairs (on xt) first so the
            # xto copy overlaps; acc chained in place
            acc = acc_pool.tile([128, FLAT], F16, tag="acc_dve")

            def pair_args(i, parity):
                # parity 0: taps (j=0, j=2) on xto; parity 1: (1, 3) on xt
                if parity == 1:
                    in0 = xt[:, i * XW + 2 : i * XW + 2 + FLAT]
                    cA = w_sb[:, i * KW + 1 : i * KW + 2]
                else:
                    in0 = xto[:, i * XW : i * XW + FLAT]
                    cA = w_sb[:, i * KW : i * KW + 1]
                cB = w_cb[:, i * 2 + parity : i * 2 + parity + 1]
                return in0, cA, cB

            order = [(i, 1) for i in range(KH)] + [(i, 0) for i in range(KH)]
            (i0, p0) = order[0]
            in0, cA, cB = pair_args(i0, p0)
            # init reads in1 but ignores it (keeps the op two-source so the
            # RTL mode detector stays at 2x_1P); feed it the in0 window.
            nc.vector._custom_dve(
                init_op, out=acc[:, :], in0=in0, in1=in0, s0=cA, s1=cB
            )
            for (i, p) in order[1:]:
                in0, cA, cB = pair_args(i, p)
                nc.vector._custom_dve(
                    acc_op, out=acc[:, :], in0=in0, in1=acc[:, :],
                    s0=cA, s1=cB,
                )

            # ---- transpose back: per (y, x4) PE transpose of stride-4
            # acc slices; ACT copy casts fp16 -> f32 into ost [q,(hb,y,x4c)]
            ost = ost_pool.tile([128, HBLK * YT * 128], F32, tag="o32")
            ostv = ost[:].rearrange(
                "q (hb y f) -> q hb y f", hb=HBLK, y=YT, f=128
            )
            ost4 = ost[:].rearrange(
                "q (hb y x4 c) -> q hb y x4 c", hb=HBLK, y=YT, x4=X4, c=C
            )
            for y in range(YT):
                ps = pso_pool.tile([128, X4 * 128], F16)  # half a PSUM bank
                av = acc[:, y * XW + XOFF : y * XW + XOFF + W].rearrange(
                    "p (q x4) -> p x4 q", x4=X4
                )
                for x4 in range(X4):
                    nc.tensor.matmul(
                        ps[:, 128 * x4 : 128 * (x4 + 1)],
                        av[:, x4],
                        id16[:],
                        start=(x4 == 0),
                        stop=(x4 == X4 - 1),
                        is_transpose=True,
                        skip_group_check=True,
                    )
                src = ps[:].rearrange(
                    "q (x4 hb c) -> q hb x4 c", x4=X4, hb=HBLK, c=C
                )
                nc.scalar.copy(ost4[:, :, y], src)
            for hb in range(HBLK):
                nc.sync.dma_start(
                    out=o_r[:, y0 : y0 + YT, hb], in_=ostv[:, hb]
                )

        for p in (pso_pool, psi_pool, ost_pool, acc_pool,
                  xbuf_pool, pre32_pool, consts):
            p.release()

    # Enable the 2x perf slot on the real instructions (byte-36[7:6]).
    for bb in nc.m.functions[0].blocks:
        for i in bb.instructions:
            if type(i).__name__ == "InstCustomDveAnt":
                i.perf_max = 1
    nc.compile()
    return nc


_CACHED = {}


def _get_program(repeat: int = 1):
    if repeat not in _CACHED:
        _CACHED[repeat] = _build_program(repeat)
    return _CACHED[repeat]


def kernel(x: np.ndarray, w: np.ndarray, _trace: bool = False,
           _repeat: int = 1):
    """Full inputs in, full output out. Shards batch across 8 cores."""
    x = np.ascontiguousarray(np.asarray(x), dtype=np.float32)
    w = np.ascontiguousarray(np.asarray(w), dtype=np.float32)
    assert x.shape == (B, H, W, C) and w.shape == (KH, KW, C)
    nc = _get_program(_repeat)
    core_ids = list(range(B))
    in_maps = [{"x": x[b], "w": w} for b in range(B)]
    res = run_bass_kernel_spmd(nc, in_maps, core_ids, trace=_trace)
    out = np.stack([res.results[i]["out"] for i in range(B)], axis=0)
    if _trace:
        kernel.last_exec_time_ns = res.exec_time_ns
        kernel.last_results = res
    return out


if __name__ == "__main__":
    rng = np.random.default_rng(0)
    x = rng.standard_normal((B, H, W, C), dtype=np.float32)
    w = (rng.standard_normal((KH, KW, C)) * 0.1).astype(np.float32)
    out = kernel(x, w)
    print("out", out.shape, out.dtype, float(out.mean()))


# revision 15
# speedup vs baseline: 5.0095x; 1.4692x over previous
"""Trainium2 Bass kernel for nn_Dilation2D (morphological dilation, max-plus conv).

    out[b,y,x,c] = max_{i,j} ( x_pad[b, y+i-1, x+j-1, c] + w[i,j,c] )

Sharding: pure data parallel over batch B=8 -> one image per NeuronCore.

Per-core layout: partitions p = hb*32 + c  (hb = one of 4 row-blocks of 128
image rows, c = channel).  Free dim = (row, x).  In this layout a tap is
    acc = max(x_tile[row+i, x+j] + w_vec[p], acc)
with w a per-partition scalar (CONST lanes), so taps fuse on the DVE.

Two structural wins over the previous version:

1. Fused PAIR-TAP custom DVE ops: one 2x_1P uop program computes TWO taps
   plus the acc max in a single pass,
       out[t] = max(in0[t-2] + cA, in0[t] + cA + cB', acc[t])
   using temporal CURR_ALU_OUT delay captures for the in0[t-2] history
   (exactly 8 ALU slices: 4 ADD + 4 MAX).  16 taps -> 8 DVE passes.
   Streams are flat [p, 8*544] full rows; the 2-element-late garbage at
   each row start lands in the 16-column left margin.

2. 512-byte DMA descriptors both directions: DRAM<->SBUF staging uses
   partitions q = x//4 so each descriptor moves (x4,c) = 4*32*4B = 512B
   of contiguous DRAM (vs 128B before, which paid the sub-512B RMW 2x
   penalty).  PE transpose tiles use f-slices (hb,c) per (r, x4); the
   PSUM->SBUF ACT copy interleaves x = 4q+x4 back into sequential x.

The (y,x,c) relayout rides the otherwise-idle TensorE (transpose-mode
matmuls), ACT does PSUM->SBUF copies (with f32<->fp16 casts), and the
one-element-shifted xto copy (for even-j tap alignment) runs on GpSimd.
"""

import numpy as np

import concourse.bass as bass
import concourse.bacc as bacc
import concourse.tile as tile
import concourse.dve_ops as dve_ops
from concourse import mybir
from concourse.bass_utils import run_bass_kernel_spmd
from concourse.dve_spec import Spec, Src0, Src1, C0, C1, maxx
from concourse.dve_uop import (
    DveOpSpec, UopConfig, UopDpConfig, InpSel, OutSel, OutPath, AluOp,
    AluInp, DelayInp, Trigger,
)

# Problem constants (hardcoded per contract).
B, H, W, C = 8, 512, 512, 32
KH, KW = 4, 4
HBLK = 4               # row blocks on partitions
HB = H // HBLK         # 128 rows per block
X4 = 4                 # x-positions per DMA descriptor (512B runs)
YT = 8                 # output rows per chunk
NCHUNK = HB // YT      # 16 chunks
RT = YT + KH - 1       # 11 input rows per chunk (with halo)
XOFF = 16              # x=0 lives at column 16
XW = 544               # padded row width: [15]=x=-1, [16,528)=x, [528,530)=halo
FLAT = YT * XW         # flat acc stream length (4352, even)
XTN = RT * XW + 8      # xt/xto alloc incl. tail pad read by the i=3 window
NEG = -60000.0         # -inf stand-in that fits fp16

F32 = mybir.dt.float32
F16 = mybir.dt.float16

_ENABLE, _DISABLE = 1, 0
_A = AluInp
_D = DelayInp


def _stage(op, a, b=None, cap=None, passes=()):
    """One datapath block: ALU = op(a, b); delay lanes in `passes` forward
    the upstream value; lanes in `cap` capture from the given DelayInp
    source (PREV_ALU_OUT = upstream flop, CURR_ALU_OUT = this slice's own
    flop from the previous element -> a one-element temporal delay)."""
    delay = [_D.PREV_ALU_OUT] * 7
    enable = [_DISABLE] * 7
    for ln in passes:
        delay[ln] = _D.PREV_DELAY
        enable[ln] = _ENABLE
    for ln, src in (cap or {}).items():
        delay[ln] = src
        enable[ln] = _ENABLE
    return UopDpConfig(
        op=op, alu_src0=a, alu_src1=b if b is not None else a,
        delay=delay, alu_out_enable=_ENABLE, swap_enable=_DISABLE,
        alu_out_a_enable=_DISABLE, alu_out_b_enable=_DISABLE,
        delay_enable=enable, idx0_sel=0, idx1_sel=0,
    )


def _uop(inp_map, dp, out_map):
    """Assemble a UopConfig. inp_map: {lane: InpSel}; out_map:
    {OutPath: OutSel} for enabled write paths."""
    inp = [InpSel.ZERO] * 8
    inp_en = [_DISABLE] * 8
    for ln, sel in inp_map.items():
        inp[ln] = sel
        inp_en[ln] = _ENABLE
    out = {p: OutSel.ALU_OUT for p in OutPath}
    out_en = {p: _DISABLE for p in OutPath}
    for p, sel in out_map.items():
        out[p] = sel
        out_en[p] = _ENABLE
    has_src1 = any(
        s in (InpSel.SRC_1, InpSel.SRC_1_HI) for s in inp_map.values()
    )
    return UopConfig(
        datapath_config=dp, inp=inp, inp_enable=inp_en,
        out=out, out_enable=out_en,
        require_inp0=1, require_inp1=1 if has_src1 else 0,
        trigger=(Trigger.SRC_TENSOR_DONE, Trigger.NONE, Trigger.NONE),
        next_uop=(0, 0, 0), repeat_count=0,
    )


def _acc_2x():
    """2x_1P: out pair = max(wA(prev pair), wB(this pair), acc pair)
    with wA = in0 + c0, wB = wA + c1.  4 ADD + 4 MAX = all 8 slices."""
    inp = {0: InpSel.SRC_0, 1: InpSel.SRC_0_HI, 2: InpSel.SRC_1,
           3: InpSel.SRC_1_HI, 4: InpSel.CONST_0, 5: InpSel.CONST_1}
    # lanes at stage0: PD0=hi PD1=accL PD2=accH PD3=cA PD4=cB'
    dp = [
        # flop=wA_lo(p); D5<-wA_lo(p-1)
        _stage(AluOp.ADD, _A.PREV_ALU_OUT, _A.PREV_DELAY_3,
               cap={5: _D.CURR_ALU_OUT}, passes=(0, 1, 2, 3, 4)),
        # flop=wA_hi(p); D0<-wA_hi(p-1); D3<-wA_lo(p)
        _stage(AluOp.ADD, _A.PREV_DELAY_0, _A.PREV_DELAY_3,
               cap={0: _D.CURR_ALU_OUT, 3: _D.PREV_ALU_OUT},
               passes=(1, 2, 4, 5)),
        # flop=wB_lo(p); D3<-wA_hi(p)
        _stage(AluOp.ADD, _A.PREV_DELAY_3, _A.PREV_DELAY_4,
               cap={3: _D.PREV_ALU_OUT}, passes=(0, 1, 2, 4, 5)),
        # flop=wB_hi(p); D3<-wB_lo(p)
        _stage(AluOp.ADD, _A.PREV_DELAY_3, _A.PREV_DELAY_4,
               cap={3: _D.PREV_ALU_OUT}, passes=(0, 1, 2, 5)),
        # flop=m_lo=max(wA_lo(p-1), wB_lo(p)); D3<-wB_hi(p)
        _stage(AluOp.MAX, _A.PREV_DELAY_5, _A.PREV_DELAY_3,
               cap={3: _D.PREV_ALU_OUT}, passes=(0, 1, 2)),
        # flop=m_hi=max(wA_hi(p-1), wB_hi(p)); D0<-m_lo
        _stage(AluOp.MAX, _A.PREV_DELAY_0, _A.PREV_DELAY_3,
               cap={0: _D.PREV_ALU_OUT}, passes=(1, 2)),
        # flop=out_lo=max(m_lo, accL); D0<-m_hi
        _stage(AluOp.MAX, _A.PREV_DELAY_0, _A.PREV_DELAY_1,
               cap={0: _D.PREV_ALU_OUT}, passes=(2,)),
        # flop=out_hi=max(m_hi, accH); D1<-out_lo
        _stage(AluOp.MAX, _A.PREV_DELAY_0, _A.PREV_DELAY_2,
               cap={1: _D.PREV_ALU_OUT}),
    ]
    return _uop(inp, dp, {OutPath.WR0_LO: OutSel.DELAY_1,
                          OutPath.WR0_HI: OutSel.ALU_OUT})


def _acc_1x():
    """1x: out[t] = max(wA(t-2), wB(t), acc[t])."""
    inp = {0: InpSel.SRC_0, 1: InpSel.SRC_1, 2: InpSel.CONST_0,
           3: InpSel.CONST_1}
    # lanes: PD0=acc PD1=cA PD2=cB'
    dp = [
        # flop=wA(t); D5<-wA(t-1)
        _stage(AluOp.ADD, _A.PREV_ALU_OUT, _A.PREV_DELAY_1,
               cap={5: _D.CURR_ALU_OUT}, passes=(0, 2)),
        # flop=wA(t-1); D5<-wA(t-2); D3<-wA(t)
        _stage(AluOp.BYPASS, _A.PREV_DELAY_5,
               cap={5: _D.CURR_ALU_OUT, 3: _D.PREV_ALU_OUT}, passes=(0, 2)),
        # flop=wB(t)
        _stage(AluOp.ADD, _A.PREV_DELAY_3, _A.PREV_DELAY_2, passes=(0, 5)),
        _stage(AluOp.MAX, _A.PREV_ALU_OUT, _A.PREV_DELAY_5, passes=(0,)),
        _stage(AluOp.MAX, _A.PREV_ALU_OUT, _A.PREV_DELAY_0),
        _stage(AluOp.BYPASS, _A.PREV_ALU_OUT),
        _stage(AluOp.BYPASS, _A.PREV_ALU_OUT),
        _stage(AluOp.BYPASS, _A.PREV_ALU_OUT),
    ]
    return _uop(inp, dp, {OutPath.WR0_LO: OutSel.ALU_OUT})


def _init_2x():
    """2x_1P 'init' variant: identical to _acc_2x but the final two MAXes
    are BYPASSes -- in1 is READ (keeping the op two-source so the RTL mode
    detector stays at 2x_1P; a single-src 16-bit aligned op would select
    the unpopulated 2x_2P/4x slots and crash the engine) but ignored."""
    inp = {0: InpSel.SRC_0, 1: InpSel.SRC_0_HI, 2: InpSel.SRC_1,
           3: InpSel.SRC_1_HI, 4: InpSel.CONST_0, 5: InpSel.CONST_1}
    dp = [
        _stage(AluOp.ADD, _A.PREV_ALU_OUT, _A.PREV_DELAY_3,
               cap={5: _D.CURR_ALU_OUT}, passes=(0, 1, 2, 3, 4)),
        _stage(AluOp.ADD, _A.PREV_DELAY_0, _A.PREV_DELAY_3,
               cap={0: _D.CURR_ALU_OUT, 3: _D.PREV_ALU_OUT},
               passes=(1, 2, 4, 5)),
        _stage(AluOp.ADD, _A.PREV_DELAY_3, _A.PREV_DELAY_4,
               cap={3: _D.PREV_ALU_OUT}, passes=(0, 1, 2, 4, 5)),
        _stage(AluOp.ADD, _A.PREV_DELAY_3, _A.PREV_DELAY_4,
               cap={3: _D.PREV_ALU_OUT}, passes=(0, 1, 2, 5)),
        _stage(AluOp.MAX, _A.PREV_DELAY_5, _A.PREV_DELAY_3,
               cap={3: _D.PREV_ALU_OUT}, passes=(0, 1, 2)),
        _stage(AluOp.MAX, _A.PREV_DELAY_0, _A.PREV_DELAY_3,
               cap={0: _D.PREV_ALU_OUT}, passes=(1, 2)),
        _stage(AluOp.BYPASS, _A.PREV_DELAY_0,
               cap={0: _D.PREV_ALU_OUT}, passes=(2,)),
        _stage(AluOp.BYPASS, _A.PREV_DELAY_0,
               cap={1: _D.PREV_ALU_OUT}),
    ]
    return _uop(inp, dp, {OutPath.WR0_LO: OutSel.DELAY_1,
                          OutPath.WR0_HI: OutSel.ALU_OUT})


def _init_1x():
    inp = {0: InpSel.SRC_0, 1: InpSel.SRC_1, 2: InpSel.CONST_0,
           3: InpSel.CONST_1}
    # lanes: PD0=acc(ignored) PD1=cA PD2=cB'
    dp = [
        _stage(AluOp.ADD, _A.PREV_ALU_OUT, _A.PREV_DELAY_1,
               cap={5: _D.CURR_ALU_OUT}, passes=(2,)),
        _stage(AluOp.BYPASS, _A.PREV_DELAY_5,
               cap={5: _D.CURR_ALU_OUT, 3: _D.PREV_ALU_OUT}, passes=(2,)),
        _stage(AluOp.ADD, _A.PREV_DELAY_3, _A.PREV_DELAY_2, passes=(5,)),
        _stage(AluOp.MAX, _A.PREV_ALU_OUT, _A.PREV_DELAY_5),
        _stage(AluOp.BYPASS, _A.PREV_ALU_OUT),
        _stage(AluOp.BYPASS, _A.PREV_ALU_OUT),
        _stage(AluOp.BYPASS, _A.PREV_ALU_OUT),
        _stage(AluOp.BYPASS, _A.PREV_ALU_OUT),
    ]
    return _uop(inp, dp, {OutPath.WR0_LO: OutSel.ALU_OUT})


def _sliding_ref(in0, s0, s1):
    in0 = np.asarray(in0, np.float32)
    a = in0 + s0
    b = a + s1
    ash = np.concatenate([np.full_like(a[..., :2], NEG), a[..., :-2]], -1)
    return np.maximum(ash, b)


def _register_op(name, body, ref, u1x, u2x, rd1):
    for o in dve_ops.OPS:
        if o.name == name:
            return o
    spec = Spec(body=body, reference=ref)
    row = dve_ops._CUSTOM_DVE_ROW_BASE + len(dve_ops.OPS)
    u2x.validate("v3")
    full = DveOpSpec(name=name, opcode=row, uops=[u1x], uops_2x=[u2x],
                     rd1_en=rd1, perf_max=1)
    full.validate("v3")
    op = dve_ops.DveOp(name, spec, subdim=False,
                       uops_sha={"v3": full.sha("v3")})
    dve_ops.OPS.append(op)
    dve_ops._SUB_OPCODE_FOR_NAME[name] = row
    dve_ops.CUSTOM_DVE_SPECS[name] = spec
    dve_ops._COMPILE_CACHE[(name, "v3")] = full
    return op


def _register_tap_ops():
    def _ref_acc(in0, in1, s0, s1, imm2):
        r = _sliding_ref(in0, s0, s1)
        return np.maximum(r, np.asarray(in1, np.float32).reshape(r.shape))

    def _ref_init(in0, in1, s0, s1, imm2):
        return _sliding_ref(in0, s0, s1)

    acc_op = _register_op(
        "PAIRTAP_ACC_ANT", maxx(Src0 + C0, Src1), _ref_acc,
        _acc_1x(), _acc_2x(), rd1=True,
    )
    init_op = _register_op(
        "PAIRTAP_INIT_ANT", maxx(Src0 + C0, Src1), _ref_init,
        _init_1x(), _init_2x(), rd1=True,
    )
    return acc_op, init_op


def _build_program(repeat: int = 1):
    acc_op, init_op = _register_tap_ops()
    nc = bacc.Bacc("TRN2", target_bir_lowering=False, debug=False)
    x_d = nc.dram_tensor("x", [H, W, C], F32, kind="ExternalInput").ap()
    w_d = nc.dram_tensor("w", [KH, KW, C], F32, kind="ExternalInput").ap()
    o_d = nc.dram_tensor("out", [H, W, C], F32, kind="ExternalOutput").ap()

    # Input view: q = x within a 128-wide block (baseline layout), so the
    # per-(r,xc) transpose f-slice (hb,c) is contiguous.  The f32->fp16
    # cast rides the SWDGE DMA (gpsimd), halving SBUF-side bytes.
    x_v = x_d.rearrange("(hb y) (xc q) c -> q y xc hb c", hb=HBLK, q=128)
    # Output view: q = x//4 on partitions -> DRAM descriptors of
    # (x4,c) = 512B, fed by stride-4 PE transposes of acc.
    o_r = o_d.rearrange("(hb y) (q xf) c -> q y hb (xf c)", hb=HBLK, xf=X4)

    with tile.TileContext(nc) as tc:
        consts = tc.alloc_tile_pool(name="consts", bufs=1)
        w_sb = consts.tile([128, KH * KW], F32)
        w_r = w_d.rearrange("i j c -> c (i j)")
        for hb in range(HBLK):
            nc.sync.dma_start(out=w_sb[32 * hb : 32 * (hb + 1), :], in_=w_r)
        # cB' = w[i][jp+2] - w[i][jp] per (i, parity jp): col i*2+jp.
        w_cb = consts.tile([128, KH * 2], F32)
        wv = w_sb[:].rearrange("p (i t) -> p i t", i=KH, t=KW)
        cbv = w_cb[:].rearrange("p (i jp) -> p i jp", i=KH, jp=2)
        nc.vector.tensor_sub(cbv, wv[:, :, 2:4], wv[:, :, 0:2])
        import concourse.masks as masks
        id32 = consts.tile([128, 128], F32)
        masks.make_identity(nc, id32[:])
        id16 = consts.tile([128, 128], F16)
        masks.make_identity(nc, id16[:])

        pre32_pool = tc.alloc_tile_pool(name="pre32", bufs=2)
        xbuf_pool = tc.alloc_tile_pool(name="xbuf", bufs=2)
        acc_pool = tc.alloc_tile_pool(name="acc", bufs=2)
        ost_pool = tc.alloc_tile_pool(name="ost", bufs=2)
        psi_pool = tc.alloc_tile_pool(name="psi", bufs=3, space="PSUM")
        pso_pool = tc.alloc_tile_pool(name="pso", bufs=3, space="PSUM")

        xt_prev = xto_prev = None
        for ck_rep in range(NCHUNK * repeat):
            ck = ck_rep % NCHUNK
            y0 = ck * YT  # first output row (within each hb block)

            # ---- load: DRAM f32 -> pre16 fp16 [q, (r, xc, hb, c)] via
            # SWDGE cast DMA (one per hb).  Rows 0..2 (the 3-row halo) are
            # carried over from the previous chunk's xt/xto instead of
            # being re-read, except at ck==0.
            r_lo = 1 if ck == 0 else 3
            r_hi = RT - 2 if ck == NCHUNK - 1 else RT
            XC = W // 128
            pre16 = pre32_pool.tile([128, RT * XC * HBLK * C], F16)
            p16v = pre16[:].rearrange(
                "q (r xc hb c) -> q r xc hb c", r=RT, xc=XC, hb=HBLK, c=C
            )
            for hb in range(HBLK):
                nc.gpsimd.dma_start(
                    out=p16v[:, r_lo:r_hi, :, hb],
                    in_=x_v[:, y0 - 1 + r_lo : y0 - 1 + r_hi, :, hb],
                )
                if ck == 0:
                    # r=0 is y_loc=-1: row 127 of block hb-1 (hb=0 gets a
                    # dummy row; masked to NEG after the relayout)
                    nc.gpsimd.dma_start(
                        out=p16v[:, 0, :, hb],
                        in_=x_v[:, HB - 1 if hb else 0, :, max(hb - 1, 0)],
                    )
                if ck == NCHUNK - 1:
                    # r in {RT-2, RT-1} are y_loc {128,129}: rows 0,1 of hb+1
                    nc.gpsimd.dma_start(
                        out=p16v[:, RT - 2 : RT, :, hb],
                        in_=x_v[:, 0:2, :, min(hb + 1, HBLK - 1)],
                    )

            # ---- relayout fresh rows: per (r, xc) fp16 PE transpose of
            # [q, (hb c)] tiles; ACT copies PSUM -> xt rows
            t_lo = 0 if ck == 0 else 3
            xt = xbuf_pool.tile([128, XTN], F16, tag="xt")
            xto = xbuf_pool.tile([128, XTN], F16, tag="xto")
            if ck != 0:
                # halo rows 0..2 <- previous chunk's rows 8..10 (incl. NEG
                # margins).  xt via SBUF->SBUF DMA, xto via GpSimd copy --
                # ACT and DVE are the loaded engines.
                nc.sync.dma_start(
                    out=xt[:, 0 : 3 * XW], in_=xt_prev[:, 8 * XW : RT * XW]
                )
                nc.gpsimd.tensor_copy(
                    xto[:, 0 : 3 * XW], xto_prev[:, 8 * XW : RT * XW]
                )
            for r in range(t_lo, RT):
                ps = psi_pool.tile([128, XC * 128], F16)  # half a PSUM bank
                for xc in range(XC):
                    nc.tensor.matmul(
                        ps[:, 128 * xc : 128 * (xc + 1)],
                        p16v[:, r, xc],
                        id16[:],
                        start=(xc == 0),
                        stop=(xc == XC - 1),
                        is_transpose=True,
                        skip_group_check=True,
                    )
                nc.scalar.copy(xt[:, r * XW + XOFF : r * XW + XOFF + W], ps[:])

            # borders on fresh rows: left halo col (x=-1), right [528, 544)
            xt_v = xt[:, : RT * XW].rearrange("p (r x) -> p r x", r=RT, x=XW)
            nc.gpsimd.memset(xt_v[:, t_lo:, XOFF - 1 : XOFF], NEG)
            nc.gpsimd.memset(xt_v[:, t_lo:, XOFF + W :], NEG)
            if ck == 0:
                nc.gpsimd.memset(xt_v[0:32, 0, :], NEG)          # hb=0, y=-1
            if ck == NCHUNK - 1:
                nc.gpsimd.memset(xt_v[96:128, RT - 2 : RT, :], NEG)  # hb=3

            # one-element-left-shifted copy of the fresh rows for even-j
            # tap pairs (ACT is alignment-blind)
            nc.scalar.copy(
                xto[:, t_lo * XW : RT * XW + 4],
                xt[:, t_lo * XW + 1 : RT * XW + 5],
            )
            xt_prev, xto_prev = xt, xto

            # ---- taps: 8 fused pair ops, odd-j pairs (on xt) first so the
            # xto copy overlaps; acc chained in place
            acc = acc_pool.tile([128, FLAT], F16, tag="acc_dve")

            def pair_args(i, parity):
                # parity 0: taps (j=0, j=2) on xto; parity 1: (1, 3) on xt
                if parity == 1:
                    in0 = xt[:, i * XW + 2 : i * XW + 2 + FLAT]
                    cA = w_sb[:, i * KW + 1 : i * KW + 2]
                else:
                    in0 = xto[:, i * XW : i * XW + FLAT]
                    cA = w_sb[:, i * KW : i * KW + 1]
                cB = w_cb[:, i * 2 + parity : i * 2 + parity + 1]
                return in0, cA, cB

            order = [(i, 1) for i in range(KH)] + [(i, 0) for i in range(KH)]
            (i0, p0) = order[0]
            in0, cA, cB = pair_args(i0, p0)
            # init reads in1 but ignores it (keeps the op two-source so the
            # RTL mode detector stays at 2x_1P); feed it the in0 window.
            nc.vector._custom_dve(
                init_op, out=acc[:, :], in0=in0, in1=in0, s0=cA, s1=cB
            )
            for (i, p) in order[1:]:
                in0, cA, cB = pair_args(i, p)
                nc.vector._custom_dve(
                    acc_op, out=acc[:, :], in0=in0, in1=acc[:, :],
                    s0=cA, s1=cB,
                )

            # ---- transpose back: per (y, x4) PE transpose of stride-4
            # acc slices; ACT copy casts fp16 -> f32 into ost [q,(hb,y,x4c)]
            ost = ost_pool.tile([128, HBLK * YT * 128], F32, tag="o32")
            ostv = ost[:].rearrange(
                "q (hb y f) -> q hb y f", hb=HBLK, y=YT, f=128
            )
            ost4 = ost[:].rearrange(
                "q (hb y x4 c) -> q hb y x4 c", hb=HBLK, y=YT, x4=X4, c=C
            )
            for y in range(YT):
                ps = pso_pool.tile([128, X4 * 128], F16)  # half a PSUM bank
                av = acc[:, y * XW + XOFF : y * XW + XOFF + W].rearrange(
                    "p (q x4) -> p x4 q", x4=X4
                )
                for x4 in range(X4):
                    nc.tensor.matmul(
                        ps[:, 128 * x4 : 128 * (x4 + 1)],
                        av[:, x4],
                        id16[:],
                        start=(x4 == 0),
                        stop=(x4 == X4 - 1),
                        is_transpose=True,
                        skip_group_check=True,
                    )
                src = ps[:].rearrange(
                    "q (x4 hb c) -> q hb x4 c", x4=X4, hb=HBLK, c=C
                )
                nc.scalar.copy(ost4[:, :, y], src)
            for hb in range(HBLK):
                nc.sync.dma_start(
                    out=o_r[:, y0 : y0 + YT, hb], in_=ostv[:, hb]
                )

        for p in (pso_pool, psi_pool, ost_pool, acc_pool,
                  xbuf_pool, pre32_pool, consts):
            p.release()

    # Enable the 2x perf slot on the real instructions (byte-36[7:6]).
    for bb in nc.m.functions[0].blocks:
        for i in bb.instructions:
            if type(i).__name__ == "InstCustomDveAnt":
                i.perf_max = 1
    nc.compile()
    return nc


_CACHED = {}


def _get_program(repeat: int = 1):
    if repeat not in _CACHED:
        _CACHED[repeat] = _build_program(repeat)
    return _CACHED[repeat]


def kernel(x: np.ndarray, w: np.ndarray, _trace: bool = False,
           _repeat: int = 1):
    """Full inputs in, full output out. Shards batch across 8 cores."""
    x = np.ascontiguousarray(np.asarray(x), dtype=np.float32)
    w = np.ascontiguousarray(np.asarray(w), dtype=np.float32)
    assert x.shape == (B, H, W, C) and w.shape == (KH, KW, C)
    nc = _get_program(_repeat)
    core_ids = list(range(B))
    in_maps = [{"x": x[b], "w": w} for b in range(B)]
    res = run_bass_kernel_spmd(nc, in_maps, core_ids, trace=_trace)
    out = np.stack([res.results[i]["out"] for i in range(B)], axis=0)
    if _trace:
        kernel.last_exec_time_ns = res.exec_time_ns
        kernel.last_results = res
    return out


if __name__ == "__main__":
    rng = np.random.default_rng(0)
    x = rng.standard_normal((B, H, W, C), dtype=np.float32)
    w = (rng.standard_normal((KH, KW, C)) * 0.1).astype(np.float32)
    out = kernel(x, w)
    print("out", out.shape, out.dtype, float(out.mean()))


# revision 17
# speedup vs baseline: 6.4936x; 1.2962x over previous
"""Trainium2 Bass kernel for nn_Dilation2D (morphological dilation, max-plus conv).

    out[b,y,x,c] = max_{i,j} ( x_pad[b, y+i-1, x+j-1, c] + w[i,j,c] )

Sharding: pure data parallel over batch B=8 -> one image per NeuronCore.

Per-core layout: partitions p = hb*32 + c  (hb = one of 4 row-blocks of 128
image rows, c = channel).  Free dim = (row, x).  In this layout a tap is
    acc = max(x_tile[row+i, x+j] + w_vec[p], acc)
with w a per-partition scalar (CONST lanes), so taps fuse on the DVE.

Two structural wins over the previous version:

1. Fused PAIR-TAP custom DVE ops: one 2x_1P uop program computes TWO taps
   plus the acc max in a single pass,
       out[t] = max(in0[t-2] + cA, in0[t] + cA + cB', acc[t])
   using temporal CURR_ALU_OUT delay captures for the in0[t-2] history
   (exactly 8 ALU slices: 4 ADD + 4 MAX).  16 taps -> 8 DVE passes.
   Streams are flat [p, 8*544] full rows; the 2-element-late garbage at
   each row start lands in the 16-column left margin.

2. 512-byte DMA descriptors both directions: DRAM<->SBUF staging uses
   partitions q = x//4 so each descriptor moves (x4,c) = 4*32*4B = 512B
   of contiguous DRAM (vs 128B before, which paid the sub-512B RMW 2x
   penalty).  PE transpose tiles use f-slices (hb,c) per (r, x4); the
   PSUM->SBUF ACT copy interleaves x = 4q+x4 back into sequential x.

The (y,x,c) relayout rides the otherwise-idle TensorE (transpose-mode
matmuls), ACT does PSUM->SBUF copies (with f32<->fp16 casts), and the
one-element-shifted xto copy (for even-j tap alignment) runs on GpSimd.
"""

import numpy as np

import concourse.bass as bass
import concourse.bacc as bacc
import concourse.tile as tile
import concourse.dve_ops as dve_ops
from concourse import mybir
from concourse.bass_utils import run_bass_kernel_spmd
from concourse.dve_spec import Spec, Src0, Src1, C0, C1, maxx
from concourse.dve_uop import (
    DveOpSpec, UopConfig, UopDpConfig, InpSel, OutSel, OutPath, AluOp,
    AluInp, DelayInp, Trigger,
)

# Problem constants (hardcoded per contract).
B, H, W, C = 8, 512, 512, 32
KH, KW = 4, 4
HBLK = 4               # row blocks on partitions
HB = H // HBLK         # 128 rows per block
X4 = 4                 # x-positions per DMA descriptor (512B runs)
YT = 8                 # output rows per chunk
NCHUNK = HB // YT      # 16 chunks
RT = YT + KH - 1       # 11 input rows per chunk (with halo)
XOFF = 2               # x=0 lives at column 2 (2-col margin absorbs the
                       # sliding-window prelude garbage exactly)
XW = 516               # row pitch: [1]=x=-1, [2,514)=x, [514,516)=halo
FLAT = YT * XW         # flat acc stream length (4128, even)
XTN = RT * XW + 8      # xt/xto alloc incl. tail pad read by the i=3 window
NEG = -60000.0         # -inf stand-in that fits fp16

F32 = mybir.dt.float32
F16 = mybir.dt.float16

_ENABLE, _DISABLE = 1, 0
_A = AluInp
_D = DelayInp


def _stage(op, a, b=None, cap=None, passes=()):
    """One datapath block: ALU = op(a, b); delay lanes in `passes` forward
    the upstream value; lanes in `cap` capture from the given DelayInp
    source (PREV_ALU_OUT = upstream flop, CURR_ALU_OUT = this slice's own
    flop from the previous element -> a one-element temporal delay)."""
    delay = [_D.PREV_ALU_OUT] * 7
    enable = [_DISABLE] * 7
    for ln in passes:
        delay[ln] = _D.PREV_DELAY
        enable[ln] = _ENABLE
    for ln, src in (cap or {}).items():
        delay[ln] = src
        enable[ln] = _ENABLE
    return UopDpConfig(
        op=op, alu_src0=a, alu_src1=b if b is not None else a,
        delay=delay, alu_out_enable=_ENABLE, swap_enable=_DISABLE,
        alu_out_a_enable=_DISABLE, alu_out_b_enable=_DISABLE,
        delay_enable=enable, idx0_sel=0, idx1_sel=0,
    )


def _uop(inp_map, dp, out_map):
    """Assemble a UopConfig. inp_map: {lane: InpSel}; out_map:
    {OutPath: OutSel} for enabled write paths."""
    inp = [InpSel.ZERO] * 8
    inp_en = [_DISABLE] * 8
    for ln, sel in inp_map.items():
        inp[ln] = sel
        inp_en[ln] = _ENABLE
    out = {p: OutSel.ALU_OUT for p in OutPath}
    out_en = {p: _DISABLE for p in OutPath}
    for p, sel in out_map.items():
        out[p] = sel
        out_en[p] = _ENABLE
    has_src1 = any(
        s in (InpSel.SRC_1, InpSel.SRC_1_HI) for s in inp_map.values()
    )
    return UopConfig(
        datapath_config=dp, inp=inp, inp_enable=inp_en,
        out=out, out_enable=out_en,
        require_inp0=1, require_inp1=1 if has_src1 else 0,
        trigger=(Trigger.SRC_TENSOR_DONE, Trigger.NONE, Trigger.NONE),
        next_uop=(0, 0, 0), repeat_count=0,
    )


def _acc_2x():
    """2x_1P: out pair = max(wA(prev pair), wB(this pair), acc pair)
    with wA = in0 + c0, wB = wA + c1.  4 ADD + 4 MAX = all 8 slices."""
    inp = {0: InpSel.SRC_0, 1: InpSel.SRC_0_HI, 2: InpSel.SRC_1,
           3: InpSel.SRC_1_HI, 4: InpSel.CONST_0, 5: InpSel.CONST_1}
    # lanes at stage0: PD0=hi PD1=accL PD2=accH PD3=cA PD4=cB'
    dp = [
        # flop=wA_lo(p); D5<-wA_lo(p-1)
        _stage(AluOp.ADD, _A.PREV_ALU_OUT, _A.PREV_DELAY_3,
               cap={5: _D.CURR_ALU_OUT}, passes=(0, 1, 2, 3, 4)),
        # flop=wA_hi(p); D0<-wA_hi(p-1); D3<-wA_lo(p)
        _stage(AluOp.ADD, _A.PREV_DELAY_0, _A.PREV_DELAY_3,
               cap={0: _D.CURR_ALU_OUT, 3: _D.PREV_ALU_OUT},
               passes=(1, 2, 4, 5)),
        # flop=wB_lo(p); D3<-wA_hi(p)
        _stage(AluOp.ADD, _A.PREV_DELAY_3, _A.PREV_DELAY_4,
               cap={3: _D.PREV_ALU_OUT}, passes=(0, 1, 2, 4, 5)),
        # flop=wB_hi(p); D3<-wB_lo(p)
        _stage(AluOp.ADD, _A.PREV_DELAY_3, _A.PREV_DELAY_4,
               cap={3: _D.PREV_ALU_OUT}, passes=(0, 1, 2, 5)),
        # flop=m_lo=max(wA_lo(p-1), wB_lo(p)); D3<-wB_hi(p)
        _stage(AluOp.MAX, _A.PREV_DELAY_5, _A.PREV_DELAY_3,
               cap={3: _D.PREV_ALU_OUT}, passes=(0, 1, 2)),
        # flop=m_hi=max(wA_hi(p-1), wB_hi(p)); D0<-m_lo
        _stage(AluOp.MAX, _A.PREV_DELAY_0, _A.PREV_DELAY_3,
               cap={0: _D.PREV_ALU_OUT}, passes=(1, 2)),
        # flop=out_lo=max(m_lo, accL); D0<-m_hi
        _stage(AluOp.MAX, _A.PREV_DELAY_0, _A.PREV_DELAY_1,
               cap={0: _D.PREV_ALU_OUT}, passes=(2,)),
        # flop=out_hi=max(m_hi, accH); D1<-out_lo
        _stage(AluOp.MAX, _A.PREV_DELAY_0, _A.PREV_DELAY_2,
               cap={1: _D.PREV_ALU_OUT}),
    ]
    return _uop(inp, dp, {OutPath.WR0_LO: OutSel.DELAY_1,
                          OutPath.WR0_HI: OutSel.ALU_OUT})


def _acc_1x():
    """1x: out[t] = max(wA(t-2), wB(t), acc[t])."""
    inp = {0: InpSel.SRC_0, 1: InpSel.SRC_1, 2: InpSel.CONST_0,
           3: InpSel.CONST_1}
    # lanes: PD0=acc PD1=cA PD2=cB'
    dp = [
        # flop=wA(t); D5<-wA(t-1)
        _stage(AluOp.ADD, _A.PREV_ALU_OUT, _A.PREV_DELAY_1,
               cap={5: _D.CURR_ALU_OUT}, passes=(0, 2)),
        # flop=wA(t-1); D5<-wA(t-2); D3<-wA(t)
        _stage(AluOp.BYPASS, _A.PREV_DELAY_5,
               cap={5: _D.CURR_ALU_OUT, 3: _D.PREV_ALU_OUT}, passes=(0, 2)),
        # flop=wB(t)
        _stage(AluOp.ADD, _A.PREV_DELAY_3, _A.PREV_DELAY_2, passes=(0, 5)),
        _stage(AluOp.MAX, _A.PREV_ALU_OUT, _A.PREV_DELAY_5, passes=(0,)),
        _stage(AluOp.MAX, _A.PREV_ALU_OUT, _A.PREV_DELAY_0),
        _stage(AluOp.BYPASS, _A.PREV_ALU_OUT),
        _stage(AluOp.BYPASS, _A.PREV_ALU_OUT),
        _stage(AluOp.BYPASS, _A.PREV_ALU_OUT),
    ]
    return _uop(inp, dp, {OutPath.WR0_LO: OutSel.ALU_OUT})


def _init_2x():
    """2x_1P 'init' variant: identical to _acc_2x but the final two MAXes
    are BYPASSes -- in1 is READ (keeping the op two-source so the RTL mode
    detector stays at 2x_1P; a single-src 16-bit aligned op would select
    the unpopulated 2x_2P/4x slots and crash the engine) but ignored."""
    inp = {0: InpSel.SRC_0, 1: InpSel.SRC_0_HI, 2: InpSel.SRC_1,
           3: InpSel.SRC_1_HI, 4: InpSel.CONST_0, 5: InpSel.CONST_1}
    dp = [
        _stage(AluOp.ADD, _A.PREV_ALU_OUT, _A.PREV_DELAY_3,
               cap={5: _D.CURR_ALU_OUT}, passes=(0, 1, 2, 3, 4)),
        _stage(AluOp.ADD, _A.PREV_DELAY_0, _A.PREV_DELAY_3,
               cap={0: _D.CURR_ALU_OUT, 3: _D.PREV_ALU_OUT},
               passes=(1, 2, 4, 5)),
        _stage(AluOp.ADD, _A.PREV_DELAY_3, _A.PREV_DELAY_4,
               cap={3: _D.PREV_ALU_OUT}, passes=(0, 1, 2, 4, 5)),
        _stage(AluOp.ADD, _A.PREV_DELAY_3, _A.PREV_DELAY_4,
               cap={3: _D.PREV_ALU_OUT}, passes=(0, 1, 2, 5)),
        _stage(AluOp.MAX, _A.PREV_DELAY_5, _A.PREV_DELAY_3,
               cap={3: _D.PREV_ALU_OUT}, passes=(0, 1, 2)),
        _stage(AluOp.MAX, _A.PREV_DELAY_0, _A.PREV_DELAY_3,
               cap={0: _D.PREV_ALU_OUT}, passes=(1, 2)),
        _stage(AluOp.BYPASS, _A.PREV_DELAY_0,
               cap={0: _D.PREV_ALU_OUT}, passes=(2,)),
        _stage(AluOp.BYPASS, _A.PREV_DELAY_0,
               cap={1: _D.PREV_ALU_OUT}),
    ]
    return _uop(inp, dp, {OutPath.WR0_LO: OutSel.DELAY_1,
                          OutPath.WR0_HI: OutSel.ALU_OUT})


def _init_1x():
    inp = {0: InpSel.SRC_0, 1: InpSel.SRC_1, 2: InpSel.CONST_0,
           3: InpSel.CONST_1}
    # lanes: PD0=acc(ignored) PD1=cA PD2=cB'
    dp = [
        _stage(AluOp.ADD, _A.PREV_ALU_OUT, _A.PREV_DELAY_1,
               cap={5: _D.CURR_ALU_OUT}, passes=(2,)),
        _stage(AluOp.BYPASS, _A.PREV_DELAY_5,
               cap={5: _D.CURR_ALU_OUT, 3: _D.PREV_ALU_OUT}, passes=(2,)),
        _stage(AluOp.ADD, _A.PREV_DELAY_3, _A.PREV_DELAY_2, passes=(5,)),
        _stage(AluOp.MAX, _A.PREV_ALU_OUT, _A.PREV_DELAY_5),
        _stage(AluOp.BYPASS, _A.PREV_ALU_OUT),
        _stage(AluOp.BYPASS, _A.PREV_ALU_OUT),
        _stage(AluOp.BYPASS, _A.PREV_ALU_OUT),
        _stage(AluOp.BYPASS, _A.PREV_ALU_OUT),
    ]
    return _uop(inp, dp, {OutPath.WR0_LO: OutSel.ALU_OUT})


def _sliding_ref(in0, s0, s1):
    in0 = np.asarray(in0, np.float32)
    a = in0 + s0
    b = a + s1
    ash = np.concatenate([np.full_like(a[..., :2], NEG), a[..., :-2]], -1)
    return np.maximum(ash, b)


def _register_op(name, body, ref, u1x, u2x, rd1):
    for o in dve_ops.OPS:
        if o.name == name:
            return o
    spec = Spec(body=body, reference=ref)
    row = dve_ops._CUSTOM_DVE_ROW_BASE + len(dve_ops.OPS)
    u2x.validate("v3")
    full = DveOpSpec(name=name, opcode=row, uops=[u1x], uops_2x=[u2x],
                     rd1_en=rd1, perf_max=1)
    full.validate("v3")
    op = dve_ops.DveOp(name, spec, subdim=False,
                       uops_sha={"v3": full.sha("v3")})
    dve_ops.OPS.append(op)
    dve_ops._SUB_OPCODE_FOR_NAME[name] = row
    dve_ops.CUSTOM_DVE_SPECS[name] = spec
    dve_ops._COMPILE_CACHE[(name, "v3")] = full
    return op


def _register_tap_ops():
    def _ref_acc(in0, in1, s0, s1, imm2):
        r = _sliding_ref(in0, s0, s1)
        return np.maximum(r, np.asarray(in1, np.float32).reshape(r.shape))

    def _ref_init(in0, in1, s0, s1, imm2):
        return _sliding_ref(in0, s0, s1)

    acc_op = _register_op(
        "PAIRTAP_ACC_ANT", maxx(Src0 + C0, Src1), _ref_acc,
        _acc_1x(), _acc_2x(), rd1=True,
    )
    init_op = _register_op(
        "PAIRTAP_INIT_ANT", maxx(Src0 + C0, Src1), _ref_init,
        _init_1x(), _init_2x(), rd1=True,
    )
    return acc_op, init_op


def _build_program(repeat: int = 1):
    acc_op, init_op = _register_tap_ops()
    nc = bacc.Bacc("TRN2", target_bir_lowering=False, debug=False)
    x_d = nc.dram_tensor("x", [H, W, C], F32, kind="ExternalInput").ap()
    w_d = nc.dram_tensor("w", [KH, KW, C], F32, kind="ExternalInput").ap()
    o_d = nc.dram_tensor("out", [H, W, C], F32, kind="ExternalOutput").ap()

    # Input view: q = x within a 128-wide block (baseline layout), so the
    # per-(r,xc) transpose f-slice (hb,c) is contiguous.  The f32->fp16
    # cast rides the SWDGE DMA (gpsimd), halving SBUF-side bytes.
    x_v = x_d.rearrange("(hb y) (xc q) c -> q y xc hb c", hb=HBLK, q=128)
    # Output view: q = x//4 on partitions -> DRAM descriptors of
    # (x4,c) = 512B, fed by stride-4 PE transposes of acc.
    o_r = o_d.rearrange("(hb y) (q xf) c -> q y hb (xf c)", hb=HBLK, xf=X4)

    with tile.TileContext(nc) as tc:
        consts = tc.alloc_tile_pool(name="consts", bufs=1)
        w_sb = consts.tile([128, KH * KW], F32)
        w_r = w_d.rearrange("i j c -> c (i j)")
        for hb in range(HBLK):
            nc.sync.dma_start(out=w_sb[32 * hb : 32 * (hb + 1), :], in_=w_r)
        # cB' = w[i][jp+2] - w[i][jp] per (i, parity jp): col i*2+jp.
        w_cb = consts.tile([128, KH * 2], F32)
        wv = w_sb[:].rearrange("p (i t) -> p i t", i=KH, t=KW)
        cbv = w_cb[:].rearrange("p (i jp) -> p i jp", i=KH, jp=2)
        nc.vector.tensor_sub(cbv, wv[:, :, 2:4], wv[:, :, 0:2])
        import concourse.masks as masks
        id32 = consts.tile([128, 128], F32)
        masks.make_identity(nc, id32[:])
        id16 = consts.tile([128, 128], F16)
        masks.make_identity(nc, id16[:])

        pre32_pool = tc.alloc_tile_pool(name="pre32", bufs=2)
        xbuf_pool = tc.alloc_tile_pool(name="xbuf", bufs=2)
        acc_pool = tc.alloc_tile_pool(name="acc", bufs=2)
        ost_pool = tc.alloc_tile_pool(name="ost", bufs=2)
        psi_pool = tc.alloc_tile_pool(name="psi", bufs=3, space="PSUM")
        pso_pool = tc.alloc_tile_pool(name="pso", bufs=3, space="PSUM")

        xt_prev = xto_prev = None
        for ck_rep in range(NCHUNK * repeat):
            ck = ck_rep % NCHUNK
            y0 = ck * YT  # first output row (within each hb block)

            # ---- load: DRAM f32 -> pre32 [q, (r, xc, hb, c)] via HWDGE
            # (RTL descriptor gen; the f32->fp16 cast happens in the ACT
            # PSUM->SBUF copy after the transpose).  Rows 0..2 (the 3-row
            # halo) are carried over from the previous chunk's xt/xto
            # instead of being re-read, except at ck==0.
            r_lo = 1 if ck == 0 else 3
            r_hi = RT - 2 if ck == NCHUNK - 1 else RT
            XC = W // 128
            pre32 = pre32_pool.tile([128, RT * XC * HBLK * C], F32)
            p16v = pre32[:].rearrange(
                "q (r xc hb c) -> q r xc hb c", r=RT, xc=XC, hb=HBLK, c=C
            )
            for hb in range(HBLK):
                nc.sync.dma_start(
                    out=p16v[:, r_lo:r_hi, :, hb],
                    in_=x_v[:, y0 - 1 + r_lo : y0 - 1 + r_hi, :, hb],
                )
                if ck == 0:
                    # r=0 is y_loc=-1: row 127 of block hb-1 (hb=0 gets a
                    # dummy row; masked to NEG after the relayout)
                    nc.sync.dma_start(
                        out=p16v[:, 0, :, hb],
                        in_=x_v[:, HB - 1 if hb else 0, :, max(hb - 1, 0)],
                    )
                if ck == NCHUNK - 1:
                    # r in {RT-2, RT-1} are y_loc {128,129}: rows 0,1 of hb+1
                    nc.sync.dma_start(
                        out=p16v[:, RT - 2 : RT, :, hb],
                        in_=x_v[:, 0:2, :, min(hb + 1, HBLK - 1)],
                    )

            # ---- relayout fresh rows: per (r, xc) fp16 PE transpose of
            # [q, (hb c)] tiles; ACT copies PSUM -> xt rows
            t_lo = 0 if ck == 0 else 3
            xt = xbuf_pool.tile([128, XTN], F16, tag="xt")
            xto = xbuf_pool.tile([128, XTN], F16, tag="xto")
            if ck != 0:
                # halo rows 0..2 <- previous chunk's rows 8..10 (incl. NEG
                # margins).  xt via SBUF->SBUF DMA, xto via GpSimd copy --
                # ACT and DVE are the loaded engines.
                nc.sync.dma_start(
                    out=xt[:, 0 : 3 * XW], in_=xt_prev[:, 8 * XW : RT * XW]
                )
                nc.gpsimd.tensor_copy(
                    xto[:, 0 : 3 * XW], xto_prev[:, 8 * XW : RT * XW]
                )
            for r in range(t_lo, RT):
                ps = psi_pool.tile([128, XC * 128], F32)  # one PSUM bank
                for xc in range(XC):
                    nc.tensor.matmul(
                        ps[:, 128 * xc : 128 * (xc + 1)],
                        p16v[:, r, xc],
                        id32[:],
                        start=(xc == 0),
                        stop=(xc == XC - 1),
                        is_transpose=True,
                        skip_group_check=True,
                    )
                nc.scalar.copy(xt[:, r * XW + XOFF : r * XW + XOFF + W], ps[:])

            # borders on fresh rows: left halo col (x=-1), right [528, 544)
            xt_v = xt[:, : RT * XW].rearrange("p (r x) -> p r x", r=RT, x=XW)
            nc.gpsimd.memset(xt_v[:, t_lo:, XOFF - 1 : XOFF], NEG)
            nc.gpsimd.memset(xt_v[:, t_lo:, XOFF + W :], NEG)
            if ck == 0:
                nc.gpsimd.memset(xt_v[0:32, 0, :], NEG)          # hb=0, y=-1
            if ck == NCHUNK - 1:
                nc.gpsimd.memset(xt_v[96:128, RT - 2 : RT, :], NEG)  # hb=3

            # one-element-left-shifted copy of the fresh rows for even-j
            # tap pairs (ACT is alignment-blind)
            nc.scalar.copy(
                xto[:, t_lo * XW : RT * XW + 4],
                xt[:, t_lo * XW + 1 : RT * XW + 5],
            )
            xt_prev, xto_prev = xt, xto

            # ---- taps: 8 fused pair ops, odd-j pairs (on xt) first so the
            # xto copy overlaps; acc chained in place
            acc = acc_pool.tile([128, FLAT], F16, tag="acc_dve")

            def pair_args(i, parity):
                # parity 0: taps (j=0, j=2) on xto; parity 1: (1, 3) on xt
                if parity == 1:
                    in0 = xt[:, i * XW + 2 : i * XW + 2 + FLAT]
                    cA = w_sb[:, i * KW + 1 : i * KW + 2]
                else:
                    in0 = xto[:, i * XW : i * XW + FLAT]
                    cA = w_sb[:, i * KW : i * KW + 1]
                cB = w_cb[:, i * 2 + parity : i * 2 + parity + 1]
                return in0, cA, cB

            order = [(i, 1) for i in range(KH)] + [(i, 0) for i in range(KH)]
            (i0, p0) = order[0]
            in0, cA, cB = pair_args(i0, p0)
            # init reads in1 but ignores it (keeps the op two-source so the
            # RTL mode detector stays at 2x_1P); feed it the in0 window.
            nc.vector._custom_dve(
                init_op, out=acc[:, :], in0=in0, in1=in0, s0=cA, s1=cB
            )
            for (i, p) in order[1:]:
                in0, cA, cB = pair_args(i, p)
                nc.vector._custom_dve(
                    acc_op, out=acc[:, :], in0=in0, in1=acc[:, :],
                    s0=cA, s1=cB,
                )

            # ---- transpose back: per (y, x4) PE transpose of stride-4
            # acc slices; ACT copy casts fp16 -> f32 into ost [q,(hb,y,x4c)]
            ost = ost_pool.tile([128, HBLK * YT * 128], F32, tag="o32")
            ostv = ost[:].rearrange(
                "q (hb y f) -> q hb y f", hb=HBLK, y=YT, f=128
            )
            ost4 = ost[:].rearrange(
                "q (hb y x4 c) -> q hb y x4 c", hb=HBLK, y=YT, x4=X4, c=C
            )
            for y in range(YT):
                ps = pso_pool.tile([128, X4 * 128], F16)  # half a PSUM bank
                av = acc[:, y * XW + XOFF : y * XW + XOFF + W].rearrange(
                    "p (q x4) -> p x4 q", x4=X4
                )
                for x4 in range(X4):
                    nc.tensor.matmul(
                        ps[:, 128 * x4 : 128 * (x4 + 1)],
                        av[:, x4],
                        id16[:],
                        start=(x4 == 0),
                        stop=(x4 == X4 - 1),
                        is_transpose=True,
                        skip_group_check=True,
                    )
                src = ps[:].rearrange(
                    "q (x4 hb c) -> q hb x4 c", x4=X4, hb=HBLK, c=C
                )
                nc.scalar.copy(ost4[:, :, y], src)
            for hb in range(HBLK):
                nc.sync.dma_start(
                    out=o_r[:, y0 : y0 + YT, hb], in_=ostv[:, hb]
                )

        for p in (pso_pool, psi_pool, ost_pool, acc_pool,
                  xbuf_pool, pre32_pool, consts):
            p.release()

    # Enable the 2x perf slot on the real instructions (byte-36[7:6]).
    for bb in nc.m.functions[0].blocks:
        for i in bb.instructions:
            if type(i).__name__ == "InstCustomDveAnt":
                i.perf_max = 1
    nc.compile()
    return nc


_CACHED = {}


def _get_program(repeat: int = 1):
    if repeat not in _CACHED:
        _CACHED[repeat] = _build_program(repeat)
    return _CACHED[repeat]


def kernel(x: np.ndarray, w: np.ndarray, _trace: bool = False,
           _repeat: int = 1):
    """Full inputs in, full output out. Shards batch across 8 cores."""
    x = np.ascontiguousarray(np.asarray(x), dtype=np.float32)
    w = np.ascontiguousarray(np.asarray(w), dtype=np.float32)
    assert x.shape == (B, H, W, C) and w.shape == (KH, KW, C)
    nc = _get_program(_repeat)
    core_ids = list(range(B))
    in_maps = [{"x": x[b], "w": w} for b in range(B)]
    res = run_bass_kernel_spmd(nc, in_maps, core_ids, trace=_trace)
    out = np.stack([res.results[i]["out"] for i in range(B)], axis=0)
    if _trace:
        kernel.last_exec_time_ns = res.exec_time_ns
        kernel.last_results = res
    return out


if __name__ == "__main__":
    rng = np.random.default_rng(0)
    x = rng.standard_normal((B, H, W, C), dtype=np.float32)
    w = (rng.standard_normal((KH, KW, C)) * 0.1).astype(np.float32)
    out = kernel(x, w)
    print("out", out.shape, out.dtype, float(out.mean()))
